# revision 6
# baseline (speedup 1.0000x reference)
"""Masked multi-head self-attention kernel for 8 Trainium2 NeuronCores.

Full module: qkv projection -> causal softmax attention (16 heads) -> out
projection, for x[4, 2048, 1024].

Sharding: core c handles batch b = c//2 and heads h0 = (c%2)*8 .. h0+8.
QKV projection + attention are fully local to a core.  The out projection
contracts over all 16 heads' channels, so the two cores of a batch exchange
their attention outputs with a pairwise AllGather (chunked over the query
dim for overlap) and each computes half of the output columns.  Each core
returns out[b][:, half].T (transposed: [512, 2048]); the host reassembles.
"""

import math
import os
import sys

for _p in ("/opt/trn_rl_repo", "/root/.axon_site/_ro/trn_rl_repo"):
    if os.path.isdir(_p) and _p not in sys.path:
        sys.path.insert(0, _p)
        break

import numpy as np

import concourse.bass as bass
import concourse.mybir as mybir
import concourse.tile as tile
from concourse import bacc
from concourse.bass_utils import run_bass_kernel_spmd
from concourse.masks import make_identity

B, T, C, H = 4, 2048, 1024, 16
D = 64                 # head dim
NCORES = 8
HPC = H // 2           # heads per core = 8
CPC = HPC * D          # channels per core = 512
P = 128                # partitions
QB = 512               # query block
NQB = T // QB          # 4
KC = C // P            # contraction chunks for C = 8
NTT = T // P           # 16 t-tiles
SCALE = 1.0 / math.sqrt(D)

F32 = mybir.dt.float32
F32R = mybir.dt.float32r
EXP = mybir.ActivationFunctionType.Exp

_CACHE = {}


def build():
    nc = bacc.Bacc("TRN2", num_devices=NCORES, debug=False)

    x = nc.dram_tensor("x", [T, C], F32, kind="ExternalInput")
    wqkv = nc.dram_tensor("wqkv", [C, 3 * CPC], F32R, kind="ExternalInput")
    bqkv = nc.dram_tensor("bqkv", [1, 3 * CPC], F32, kind="ExternalInput")
    wout = nc.dram_tensor("wout", [C, CPC], F32R, kind="ExternalInput")
    bout = nc.dram_tensor("bout", [1, CPC], F32, kind="ExternalInput")
    outT = nc.dram_tensor("outT", [CPC, T], F32, kind="ExternalOutput")

    groups = [[0, 1], [2, 3], [4, 5], [6, 7]]

    with tile.TileContext(nc) as tc:
        with (
            tc.tile_pool(name="const", bufs=1) as constp,
            tc.tile_pool(name="ytp", bufs=1) as ytp,
            tc.tile_pool(name="vaugp", bufs=1) as vaugp,
            tc.tile_pool(name="dram", bufs=1, space="DRAM") as dramp,
        ):
            ident = constp.tile([P, P], F32, tag="ident")
            make_identity(nc, ident[:])
            # per-partition bias layouts: bq_sb[p, n] = bqkv[n*128 + p]
            bq_sb = constp.tile([P, 12], F32, tag="bq")
            nc.sync.dma_start(
                bq_sb[:].rearrange("p (o n) -> p o n", o=1),
                bqkv.ap().rearrange("o (n p) -> p o n", p=P),
            )
            bo_sb = constp.tile([P, 4], F32, tag="bo")
            nc.sync.dma_start(
                bo_sb[:].rearrange("p (o n) -> p o n", o=1),
                bout.ap().rearrange("o (n p) -> p o n", p=P),
            )

            # Q^T,K^T: chunk n in 0..7 ([128 ch, 2048 t] at cols n*2048)
            yt = ytp.tile([P, 8 * T], F32R, tag="yt")
            # V natural (+ones col): head h, ktile k at cols (h*16+k)*65
            vaug = vaugp.tile([P, HPC * NTT * 65], F32R, tag="vaug")
            vaug3 = vaug[:].rearrange("p (k c) -> p k c", c=65)
            ones_f32 = constp.tile([P, P], F32, tag="ones")
            nc.vector.memset(ones_f32[:], 1.0)
            nc.vector.tensor_copy(
                vaug3[:, :, 64:65],
                ones_f32[:].rearrange("p (a b) -> p a b", b=1),
            )

            # ---------------- stage 1: x^T, qkv projection, V ----------
            with (
                tc.tile_pool(name="xtp", bufs=1) as xtp,
                tc.tile_pool(name="xrow", bufs=3) as xrowp,
                tc.tile_pool(name="wtile", bufs=4) as wtp,
                tc.tile_pool(name="vtmp", bufs=2) as vtmpp,
                tc.tile_pool(name="ps_y", bufs=5, space="PSUM") as psy,
                tc.tile_pool(name="ps_tr", bufs=2, space="PSUM") as pstr,
            ):
                # x^T chunks: [128 ch, 2048 t] at cols cc*2048
                xt = xtp.tile([P, KC * T], F32R, tag="xt")
                xt3 = xt[:].rearrange("p (c t) -> p c t", t=T)
                for tt in range(NTT):
                    xrow = xrowp.tile([P, C], F32, tag="xrow")
                    nc.sync.dma_start(xrow[:], x[tt * P:(tt + 1) * P, :])
                    for cg in range(2):  # groups of 4 transposes per psum tile
                        ptr = pstr.tile([P, 512], F32, tag="ptr")
                        for j in range(4):
                            cc = cg * 4 + j
                            nc.tensor.transpose(
                                ptr[:, j * P:(j + 1) * P],
                                xrow[:, cc * P:(cc + 1) * P],
                                ident[:],
                            )
                        nc.vector.tensor_copy(
                            xt3[:, cg * 4:(cg + 1) * 4, tt * P:(tt + 1) * P],
                            ptr[:].rearrange("p (j t) -> p j t", t=P),
                        )

                # qkv: Yt[n, t] for n-tile 0..11 (Q 0-3, K 4-7, V 8-11)
                for n in range(12):
                    for tc4 in range(4):
                        py = psy.tile([P, QB], F32, tag="py")
                        for kc in range(KC):
                            if tc4 == 0:
                                wt = wtp.tile([P, P], F32R, tag="wt")
                                nc.sync.dma_start(
                                    wt[:],
                                    wqkv[kc * P:(kc + 1) * P, n * P:(n + 1) * P],
                                )
                                if kc == 0:
                                    wts = []
                                wts.append(wt)
                            nc.tensor.matmul(
                                py[:],
                                (wts[kc][:]),
                                (xt3[:, kc, tc4 * QB:(tc4 + 1) * QB]),
                                start=(kc == 0),
                                stop=(kc == KC - 1),
                            )
                        if n < 8:
                            # Q^T/K^T with bias (per-partition scalar add)
                            nc.vector.tensor_scalar_add(
                                yt[:, n * T + tc4 * QB: n * T + (tc4 + 1) * QB],
                                py[:],
                                bq_sb[:, n:n + 1],
                            )
                        else:
                            # V^T chunk -> transpose to natural V in vaug
                            vn = n - 8  # covers heads 2vn, 2vn+1
                            vtmp = vtmpp.tile([P, QB], F32, tag="vtmp")
                            nc.vector.tensor_scalar_add(
                                vtmp[:], py[:], bq_sb[:, n:n + 1]
                            )
                            ptr = pstr.tile([P, 512], F32, tag="ptr")
                            for j in range(4):
                                nc.tensor.transpose(
                                    ptr[:, j * P:(j + 1) * P],
                                    vtmp[:, j * P:(j + 1) * P],
                                    ident[:],
                                )
                            ptr3 = ptr[:].rearrange("p (j t) -> p j t", t=P)
                            for hh in range(2):
                                h = 2 * vn + hh
                                nc.vector.tensor_copy(
                                    vaug3[:, h * NTT + tc4 * 4: h * NTT + tc4 * 4 + 4, 0:64],
                                    ptr3[:, :, hh * 64:hh * 64 + 64],
                                )

            # ---------------- stage 2+3: attention, gather, out proj ----
            with (
                tc.tile_pool(name="pt", bufs=4) as ptp,
                tc.tile_pool(name="recip", bufs=3) as recipp,
                tc.tile_pool(name="bc", bufs=3) as bcp,
                tc.tile_pool(name="atv", bufs=3) as atvp,
                tc.tile_pool(name="w2", bufs=1) as w2p,
                tc.tile_pool(name="agr", bufs=2) as agrp,
                tc.tile_pool(name="outsb", bufs=3) as outsbp,
                tc.tile_pool(name="ps_s", bufs=3, space="PSUM") as pss,
                tc.tile_pool(name="ps_a", bufs=2, space="PSUM") as psa,
                tc.tile_pool(name="ps_o", bufs=2, space="PSUM") as pso,
            ):
                w2sb = w2p.tile([P, KC * CPC], F32R, tag="w2")
                nc.sync.dma_start(
                    w2sb[:].rearrange("p (c n) -> p c n", n=CPC),
                    wout.ap().rearrange("(c p) n -> p c n", p=P),
                )

                for qb in range(NQB):
                    ag_in = dramp.tile([CPC, QB], F32R, tag=f"agin{qb}")
                    ag_out = dramp.tile([2 * CPC, QB], F32R, tag=f"agout{qb}")
                    for h in range(HPC):
                        poff = (h % 2) * 64
                        qchunk = h // 2
                        kchunk = 4 + h // 2
                        pa = psa.tile([P, QB], F32, tag="pa")
                        nkt = 4 * qb + 4
                        for kt in range(nkt):
                            j = kt - 4 * qb  # >=0 on diagonal tiles
                            qoff = max(j, 0) * P
                            ps = pss.tile([P, QB], F32, tag="ps")
                            nc.tensor.matmul(
                                ps[:, qoff:QB],
                                (yt[poff:poff + 64,
                                      kchunk * T + kt * P: kchunk * T + (kt + 1) * P]),
                                (yt[poff:poff + 64,
                                      qchunk * T + qb * QB + qoff: qchunk * T + (qb + 1) * QB]),
                                start=True, stop=True,
                            )
                            pt = ptp.tile([P, QB], F32R, tag="pt")
                            nc.scalar.activation(
                                pt[:, qoff:QB], ps[:, qoff:QB], EXP, scale=SCALE
                            )
                            if j >= 0:
                                # zero where q < k (also fills the stale prefix)
                                nc.gpsimd.affine_select(
                                    out=pt[:],
                                    in_=pt[:],
                                    compare_op=mybir.AluOpType.is_ge,
                                    fill=0.0,
                                    base=-j * P,
                                    pattern=[[1, QB]],
                                    channel_multiplier=-1,
                                )
                            nc.tensor.matmul(
                                pa[0:65, :],
                                (vaug3[:, h * NTT + kt, :]),
                                (pt[:]),
                                start=(kt == 0),
                                stop=(kt == nkt - 1),
                            )
                        recip = recipp.tile([1, QB], F32, tag="recip")
                        nc.vector.reciprocal(recip[:], pa[64:65, :])
                        bc = bcp.tile([64, QB], F32, tag="bc")
                        nc.gpsimd.partition_broadcast(bc[:], recip[:])
                        atv = atvp.tile([64, QB], F32R, tag="atv")
                        nc.vector.tensor_mul(atv[:], pa[0:64, :], bc[:])
                        nc.sync.dma_start(ag_in[h * 64:(h + 1) * 64, :], atv[:])

                    nc.gpsimd.collective_compute(
                        "AllGather",
                        mybir.AluOpType.bypass,
                        replica_groups=groups,
                        ins=[ag_in.opt()],
                        outs=[ag_out.opt()],
                    )

                    # out projection (transposed): outT[oc, t] for this qb
                    agr = agrp.tile([P, KC * QB], F32R, tag="agr")
                    nc.sync.dma_start(
                        agr[:].rearrange("p (c n) -> p c n", n=QB),
                        ag_out[:].rearrange("(c p) n -> p c n", p=P),
                    )
                    agr3 = agr[:].rearrange("p (c n) -> p c n", n=QB)
                    w23 = w2sb[:].rearrange("p (c n) -> p c n", n=CPC)
                    for oc in range(4):
                        po = pso.tile([P, QB], F32, tag="po")
                        for cc in range(KC):
                            nc.tensor.matmul(
                                po[:],
                                (w23[:, cc, oc * P:(oc + 1) * P]),
                                (agr3[:, cc, :]),
                                start=(cc == 0),
                                stop=(cc == KC - 1),
                            )
                        osb = outsbp.tile([P, QB], F32, tag="osb")
                        nc.vector.tensor_scalar_add(
                            osb[:], po[:], bo_sb[:, oc:oc + 1]
                        )
                        nc.sync.dma_start(
                            outT[oc * P:(oc + 1) * P, qb * QB:(qb + 1) * QB],
                            osb[:],
                        )

    nc.compile()
    return nc


def kernel(x, w_qkv, b_qkv, w_out, b_out):
    x = np.ascontiguousarray(np.asarray(x, dtype=np.float32))
    w_qkv = np.asarray(w_qkv, dtype=np.float32)
    b_qkv = np.asarray(b_qkv, dtype=np.float32)
    w_out = np.asarray(w_out, dtype=np.float32)
    b_out = np.asarray(b_out, dtype=np.float32)

    if "nc" not in _CACHE:
        _CACHE["nc"] = build()
    nc = _CACHE["nc"]

    in_maps = []
    for c in range(NCORES):
        b = c // 2
        h0 = (c % 2) * HPC
        cols = slice(h0 * D, h0 * D + CPC)
        wq = np.concatenate(
            [w_qkv[:, cols], w_qkv[:, C:][:, cols], w_qkv[:, 2 * C:][:, cols]],
            axis=1,
        )
        bq = np.concatenate(
            [b_qkv[cols], b_qkv[C:][cols], b_qkv[2 * C:][cols]]
        ).reshape(1, 3 * CPC)
        half = slice((c % 2) * CPC, (c % 2) * CPC + CPC)
        in_maps.append({
            "x": np.ascontiguousarray(x[b]),
            "wqkv": np.ascontiguousarray(wq),
            "bqkv": np.ascontiguousarray(bq),
            "wout": np.ascontiguousarray(w_out[:, half]),
            "bout": np.ascontiguousarray(b_out[half]).reshape(1, CPC),
        })

    kwargs = {}
    tdir = os.environ.get("KERNEL_TRACE_DIR")
    if tdir:
        kwargs = dict(trace=True, tmpdir=tdir)
    res = run_bass_kernel_spmd(nc, in_maps, core_ids=list(range(NCORES)), **kwargs)
    _CACHE["last_results"] = res

    out = np.empty((B, T, C), dtype=np.float32)
    for c in range(NCORES):
        b = c // 2
        half = slice((c % 2) * CPC, (c % 2) * CPC + CPC)
        out[b][:, half] = res.results[c]["outT"].T
    return out


# revision 7
# speedup vs baseline: 1.0073x; 1.0073x over previous
"""Masked multi-head self-attention kernel for 8 Trainium2 NeuronCores.

Full module: qkv projection -> causal softmax attention (16 heads) -> out
projection, for x[4, 2048, 1024].

Sharding: core c handles batch b = c//2 and heads h0 = (c%2)*8 .. h0+8.
QKV projection + attention are fully local to a core.  The out projection
contracts over all 16 heads' channels, so the two cores of a batch exchange
their attention outputs with a pairwise AllGather (chunked over the query
dim for overlap) and each computes half of the output columns.  Each core
returns out[b][:, half].T (transposed: [512, 2048]); the host reassembles.
"""

import math
import os
import sys

for _p in ("/opt/trn_rl_repo", "/root/.axon_site/_ro/trn_rl_repo"):
    if os.path.isdir(_p) and _p not in sys.path:
        sys.path.insert(0, _p)
        break

import numpy as np

import concourse.bass as bass
import concourse.mybir as mybir
import concourse.tile as tile
from concourse import bacc
from concourse.bass_utils import run_bass_kernel_spmd
from concourse.masks import make_identity

B, T, C, H = 4, 2048, 1024, 16
D = 64                 # head dim
NCORES = 8
HPC = H // 2           # heads per core = 8
CPC = HPC * D          # channels per core = 512
P = 128                # partitions
QB = 512               # query block
NQB = T // QB          # 4
KC = C // P            # contraction chunks for C = 8
NTT = T // P           # 16 t-tiles
SCALE = 1.0 / math.sqrt(D)

F32 = mybir.dt.float32
F32R = mybir.dt.float32r
BF16 = mybir.dt.bfloat16
EXP = mybir.ActivationFunctionType.Exp

_CACHE = {}


def build():
    nc = bacc.Bacc("TRN2", num_devices=NCORES, debug=False)

    x = nc.dram_tensor("x", [T, C], F32, kind="ExternalInput")
    wqkv = nc.dram_tensor("wqkv", [C, 3 * CPC], F32R, kind="ExternalInput")
    bqkv = nc.dram_tensor("bqkv", [1, 3 * CPC], F32, kind="ExternalInput")
    wout = nc.dram_tensor("wout", [C, CPC], F32R, kind="ExternalInput")
    bout = nc.dram_tensor("bout", [1, CPC], F32, kind="ExternalInput")
    outT = nc.dram_tensor("outT", [CPC, T], F32, kind="ExternalOutput")

    groups = [[0, 1], [2, 3], [4, 5], [6, 7]]

    with tile.TileContext(nc) as tc:
        with (
            tc.tile_pool(name="const", bufs=1) as constp,
            tc.tile_pool(name="ytp", bufs=1) as ytp,
            tc.tile_pool(name="vaugp", bufs=1) as vaugp,
            tc.tile_pool(name="dram", bufs=1, space="DRAM") as dramp,
        ):
            ident = constp.tile([P, P], F32, tag="ident")
            make_identity(nc, ident[:])
            # per-partition bias layouts: bq_sb[p, n] = bqkv[n*128 + p]
            bq_sb = constp.tile([P, 12], F32, tag="bq")
            nc.sync.dma_start(
                bq_sb[:].rearrange("p (o n) -> p o n", o=1),
                bqkv.ap().rearrange("o (n p) -> p o n", p=P),
            )
            bo_sb = constp.tile([P, 4], F32, tag="bo")
            nc.sync.dma_start(
                bo_sb[:].rearrange("p (o n) -> p o n", o=1),
                bout.ap().rearrange("o (n p) -> p o n", p=P),
            )

            # Q^T,K^T: chunk n in 0..7 ([128 ch, 2048 t] at cols n*2048)
            yt = ytp.tile([P, 8 * T], BF16, tag="yt")
            # V natural (+ones col): head h, ktile k at cols (h*16+k)*65
            vaug = vaugp.tile([P, HPC * NTT * 65], BF16, tag="vaug")
            vaug3 = vaug[:].rearrange("p (k c) -> p k c", c=65)
            ones_f32 = constp.tile([P, P], F32, tag="ones")
            nc.vector.memset(ones_f32[:], 1.0)
            nc.vector.tensor_copy(
                vaug3[:, :, 64:65],
                ones_f32[:].rearrange("p (a b) -> p a b", b=1),
            )

            # ---------------- stage 1: x^T, qkv projection, V ----------
            with (
                tc.tile_pool(name="xtp", bufs=1) as xtp,
                tc.tile_pool(name="xrow", bufs=3) as xrowp,
                tc.tile_pool(name="wtile", bufs=4) as wtp,
                tc.tile_pool(name="vtmp", bufs=2) as vtmpp,
                tc.tile_pool(name="ps_y", bufs=5, space="PSUM") as psy,
                tc.tile_pool(name="ps_tr", bufs=2, space="PSUM") as pstr,
            ):
                # x^T chunks: [128 ch, 2048 t] at cols cc*2048
                xt = xtp.tile([P, KC * T], F32R, tag="xt")
                xt3 = xt[:].rearrange("p (c t) -> p c t", t=T)
                for tt in range(NTT):
                    xrow = xrowp.tile([P, C], F32, tag="xrow")
                    nc.sync.dma_start(xrow[:], x[tt * P:(tt + 1) * P, :])
                    for cg in range(2):  # groups of 4 transposes per psum tile
                        ptr = pstr.tile([P, 512], F32, tag="ptr")
                        for j in range(4):
                            cc = cg * 4 + j
                            nc.tensor.transpose(
                                ptr[:, j * P:(j + 1) * P],
                                xrow[:, cc * P:(cc + 1) * P],
                                ident[:],
                            )
                        nc.vector.tensor_copy(
                            xt3[:, cg * 4:(cg + 1) * 4, tt * P:(tt + 1) * P],
                            ptr[:].rearrange("p (j t) -> p j t", t=P),
                        )

                # qkv: Yt[n, t] for n-tile 0..11 (Q 0-3, K 4-7, V 8-11)
                for n in range(12):
                    for tc4 in range(4):
                        py = psy.tile([P, QB], F32, tag="py")
                        for kc in range(KC):
                            if tc4 == 0:
                                wt = wtp.tile([P, P], F32R, tag="wt")
                                nc.sync.dma_start(
                                    wt[:],
                                    wqkv[kc * P:(kc + 1) * P, n * P:(n + 1) * P],
                                )
                                if kc == 0:
                                    wts = []
                                wts.append(wt)
                            nc.tensor.matmul(
                                py[:],
                                (wts[kc][:]),
                                (xt3[:, kc, tc4 * QB:(tc4 + 1) * QB]),
                                start=(kc == 0),
                                stop=(kc == KC - 1),
                            )
                        if n < 8:
                            # Q^T/K^T with bias (per-partition scalar add)
                            nc.vector.tensor_scalar_add(
                                yt[:, n * T + tc4 * QB: n * T + (tc4 + 1) * QB],
                                py[:],
                                bq_sb[:, n:n + 1],
                            )
                        else:
                            # V^T chunk -> transpose to natural V in vaug
                            vn = n - 8  # covers heads 2vn, 2vn+1
                            vtmp = vtmpp.tile([P, QB], F32, tag="vtmp")
                            nc.vector.tensor_scalar_add(
                                vtmp[:], py[:], bq_sb[:, n:n + 1]
                            )
                            ptr = pstr.tile([P, 512], F32, tag="ptr")
                            for j in range(4):
                                nc.tensor.transpose(
                                    ptr[:, j * P:(j + 1) * P],
                                    vtmp[:, j * P:(j + 1) * P],
                                    ident[:],
                                )
                            ptr3 = ptr[:].rearrange("p (j t) -> p j t", t=P)
                            for hh in range(2):
                                h = 2 * vn + hh
                                nc.vector.tensor_copy(
                                    vaug3[:, h * NTT + tc4 * 4: h * NTT + tc4 * 4 + 4, 0:64],
                                    ptr3[:, :, hh * 64:hh * 64 + 64],
                                )

            # ---------------- stage 2+3: attention, gather, out proj ----
            with (
                tc.tile_pool(name="pt", bufs=4) as ptp,
                tc.tile_pool(name="recip", bufs=3) as recipp,
                tc.tile_pool(name="bc", bufs=3) as bcp,
                tc.tile_pool(name="atv", bufs=3) as atvp,
                tc.tile_pool(name="w2", bufs=1) as w2p,
                tc.tile_pool(name="agr", bufs=2) as agrp,
                tc.tile_pool(name="outsb", bufs=3) as outsbp,
                tc.tile_pool(name="ps_s", bufs=3, space="PSUM") as pss,
                tc.tile_pool(name="ps_a", bufs=2, space="PSUM") as psa,
                tc.tile_pool(name="ps_o", bufs=2, space="PSUM") as pso,
            ):
                w2sb = w2p.tile([P, KC * CPC], F32R, tag="w2")
                nc.sync.dma_start(
                    w2sb[:].rearrange("p (c n) -> p c n", n=CPC),
                    wout.ap().rearrange("(c p) n -> p c n", p=P),
                )

                for qb in range(NQB):
                    ag_in = dramp.tile([CPC, QB], F32R, tag=f"agin{qb}")
                    ag_out = dramp.tile([2 * CPC, QB], F32R, tag=f"agout{qb}")
                    for h in range(HPC):
                        poff = (h % 2) * 64
                        qchunk = h // 2
                        kchunk = 4 + h // 2
                        pa = psa.tile([P, QB], F32, tag="pa")
                        nkt = 4 * qb + 4
                        for kt in range(nkt):
                            j = kt - 4 * qb  # >=0 on diagonal tiles
                            qoff = max(j, 0) * P
                            ps = pss.tile([P, QB], F32, tag="ps")
                            nc.tensor.matmul(
                                ps[:, qoff:QB],
                                (yt[poff:poff + 64,
                                      kchunk * T + kt * P: kchunk * T + (kt + 1) * P]),
                                (yt[poff:poff + 64,
                                      qchunk * T + qb * QB + qoff: qchunk * T + (qb + 1) * QB]),
                                start=True, stop=True,
                            )
                            pt = ptp.tile([P, QB], BF16, tag="pt")
                            nc.scalar.activation(
                                pt[:, qoff:QB], ps[:, qoff:QB], EXP, scale=SCALE
                            )
                            if j >= 0:
                                # zero where q < k (also fills the stale prefix)
                                nc.gpsimd.affine_select(
                                    out=pt[:],
                                    in_=pt[:],
                                    compare_op=mybir.AluOpType.is_ge,
                                    fill=0.0,
                                    base=-j * P,
                                    pattern=[[1, QB]],
                                    channel_multiplier=-1,
                                )
                            nc.tensor.matmul(
                                pa[0:65, :],
                                (vaug3[:, h * NTT + kt, :]),
                                (pt[:]),
                                start=(kt == 0),
                                stop=(kt == nkt - 1),
                            )
                        recip = recipp.tile([1, QB], F32, tag="recip")
                        nc.vector.reciprocal(recip[:], pa[64:65, :])
                        bc = bcp.tile([64, QB], F32, tag="bc")
                        nc.gpsimd.partition_broadcast(bc[:], recip[:])
                        atv = atvp.tile([64, QB], F32R, tag="atv")
                        nc.vector.tensor_mul(atv[:], pa[0:64, :], bc[:])
                        nc.sync.dma_start(ag_in[h * 64:(h + 1) * 64, :], atv[:])

                    nc.gpsimd.collective_compute(
                        "AllGather",
                        mybir.AluOpType.bypass,
                        replica_groups=groups,
                        ins=[ag_in.opt()],
                        outs=[ag_out.opt()],
                    )

                    # out projection (transposed): outT[oc, t] for this qb
                    agr = agrp.tile([P, KC * QB], F32R, tag="agr")
                    nc.sync.dma_start(
                        agr[:].rearrange("p (c n) -> p c n", n=QB),
                        ag_out[:].rearrange("(c p) n -> p c n", p=P),
                    )
                    agr3 = agr[:].rearrange("p (c n) -> p c n", n=QB)
                    w23 = w2sb[:].rearrange("p (c n) -> p c n", n=CPC)
                    for oc in range(4):
                        po = pso.tile([P, QB], F32, tag="po")
                        for cc in range(KC):
                            nc.tensor.matmul(
                                po[:],
                                (w23[:, cc, oc * P:(oc + 1) * P]),
                                (agr3[:, cc, :]),
                                start=(cc == 0),
                                stop=(cc == KC - 1),
                            )
                        osb = outsbp.tile([P, QB], F32, tag="osb")
                        nc.vector.tensor_scalar_add(
                            osb[:], po[:], bo_sb[:, oc:oc + 1]
                        )
                        nc.sync.dma_start(
                            outT[oc * P:(oc + 1) * P, qb * QB:(qb + 1) * QB],
                            osb[:],
                        )

    nc.compile()
    return nc


def kernel(x, w_qkv, b_qkv, w_out, b_out):
    x = np.ascontiguousarray(np.asarray(x, dtype=np.float32))
    w_qkv = np.asarray(w_qkv, dtype=np.float32)
    b_qkv = np.asarray(b_qkv, dtype=np.float32)
    w_out = np.asarray(w_out, dtype=np.float32)
    b_out = np.asarray(b_out, dtype=np.float32)

    if "nc" not in _CACHE:
        _CACHE["nc"] = build()
    nc = _CACHE["nc"]

    in_maps = []
    for c in range(NCORES):
        b = c // 2
        h0 = (c % 2) * HPC
        cols = slice(h0 * D, h0 * D + CPC)
        wq = np.concatenate(
            [w_qkv[:, cols], w_qkv[:, C:][:, cols], w_qkv[:, 2 * C:][:, cols]],
            axis=1,
        )
        bq = np.concatenate(
            [b_qkv[cols], b_qkv[C:][cols], b_qkv[2 * C:][cols]]
        ).reshape(1, 3 * CPC)
        half = slice((c % 2) * CPC, (c % 2) * CPC + CPC)
        in_maps.append({
            "x": np.ascontiguousarray(x[b]),
            "wqkv": np.ascontiguousarray(wq),
            "bqkv": np.ascontiguousarray(bq),
            "wout": np.ascontiguousarray(w_out[:, half]),
            "bout": np.ascontiguousarray(b_out[half]).reshape(1, CPC),
        })

    kwargs = {}
    tdir = os.environ.get("KERNEL_TRACE_DIR")
    if tdir:
        kwargs = dict(trace=True, tmpdir=tdir)
    res = run_bass_kernel_spmd(nc, in_maps, core_ids=list(range(NCORES)), **kwargs)
    _CACHE["last_results"] = res

    out = np.empty((B, T, C), dtype=np.float32)
    for c in range(NCORES):
        b = c // 2
        half = slice((c % 2) * CPC, (c % 2) * CPC + CPC)
        out[b][:, half] = res.results[c]["outT"].T
    return out


# revision 10
# speedup vs baseline: 1.3061x; 1.2967x over previous
"""Masked multi-head self-attention kernel for 8 Trainium2 NeuronCores.

Full module: qkv projection -> causal softmax attention (16 heads) -> out
projection, for x[4, 2048, 1024].

Sharding: core c handles batch b = c//2 and heads h0 = (c%2)*8 .. h0+8.
QKV projection + attention are fully local to a core.  The out projection
contracts over all 16 heads' channels, so the two cores of a batch exchange
their attention outputs with a pairwise AllGather (chunked over the query
dim for overlap) and each computes half of the output columns.  Each core
returns out[b][:, half].T (transposed: [512, 2048]); the host reassembles.
"""

import math
import os
import sys

for _p in ("/opt/trn_rl_repo", "/root/.axon_site/_ro/trn_rl_repo"):
    if os.path.isdir(_p) and _p not in sys.path:
        sys.path.insert(0, _p)
        break

import numpy as np

import concourse.bass as bass
import concourse.mybir as mybir
import concourse.tile as tile
from concourse import bacc
from concourse.bass_utils import run_bass_kernel_spmd
from concourse.masks import make_identity

B, T, C, H = 4, 2048, 1024, 16
D = 64                 # head dim
NCORES = 8
HPC = H // 2           # heads per core = 8
CPC = HPC * D          # channels per core = 512
P = 128                # partitions
QB = 512               # query block
NQB = T // QB          # 4
KC = C // P            # contraction chunks for C = 8
NTT = T // P           # 16 t-tiles
SCALE = 1.0 / math.sqrt(D)

F32 = mybir.dt.float32
F32R = mybir.dt.float32r
BF16 = mybir.dt.bfloat16
EXP = mybir.ActivationFunctionType.Exp

_CACHE = {}


def build():
    nc = bacc.Bacc("TRN2", num_devices=NCORES, debug=False)

    x = nc.dram_tensor("x", [T, C], F32, kind="ExternalInput")
    wqkv = nc.dram_tensor("wqkv", [C, 3 * CPC], F32R, kind="ExternalInput")
    bqkv = nc.dram_tensor("bqkv", [1, 3 * CPC], F32, kind="ExternalInput")
    wout = nc.dram_tensor("wout", [C, CPC], F32R, kind="ExternalInput")
    bout = nc.dram_tensor("bout", [1, CPC], F32, kind="ExternalInput")
    outT = nc.dram_tensor("outT", [CPC, T], F32, kind="ExternalOutput")

    groups = [[0, 1], [2, 3], [4, 5], [6, 7]]

    with tile.TileContext(nc) as tc:
        with (
            tc.tile_pool(name="const", bufs=1) as constp,
            tc.tile_pool(name="ytp", bufs=1) as ytp,
            tc.tile_pool(name="vaugp", bufs=1) as vaugp,
            tc.tile_pool(name="dram", bufs=1, space="DRAM") as dramp,
        ):
            ident = constp.tile([P, P], F32, tag="ident")
            make_identity(nc, ident[:])
            # per-partition bias layouts: bq_sb[p, n] = bqkv[n*128 + p]
            bq_sb = constp.tile([P, 12], F32, tag="bq")
            nc.sync.dma_start(
                bq_sb[:].rearrange("p (o n) -> p o n", o=1),
                bqkv.ap().rearrange("o (n p) -> p o n", p=P),
            )
            bo_sb = constp.tile([P, 4], F32, tag="bo")
            nc.sync.dma_start(
                bo_sb[:].rearrange("p (o n) -> p o n", o=1),
                bout.ap().rearrange("o (n p) -> p o n", p=P),
            )

            # Q^T,K^T: chunk n in 0..7 ([128 ch, 2048 t] at cols n*2048)
            yt = ytp.tile([P, 8 * T], BF16, tag="yt")
            # V natural (+ones col): head h, ktile k at cols (h*16+k)*65
            vaug = vaugp.tile([P, HPC * NTT * 65], BF16, tag="vaug")
            vaug3 = vaug[:].rearrange("p (k c) -> p k c", c=65)
            ones_f32 = constp.tile([P, P], F32, tag="ones")
            nc.vector.memset(ones_f32[:], 1.0)
            nc.vector.tensor_copy(
                vaug3[:, :, 64:65],
                ones_f32[:].rearrange("p (a b) -> p a b", b=1),
            )

            # ---------------- stage 1: x^T, qkv projection, V ----------
            with (
                tc.tile_pool(name="xtp", bufs=1) as xtp,
                tc.tile_pool(name="xrow", bufs=3) as xrowp,
                tc.tile_pool(name="wtile", bufs=4) as wtp,
                tc.tile_pool(name="vtmp", bufs=2) as vtmpp,
                tc.tile_pool(name="ps_y", bufs=5, space="PSUM") as psy,
                tc.tile_pool(name="ps_tr", bufs=2, space="PSUM") as pstr,
            ):
                # x^T chunks: [128 ch, 2048 t] at cols cc*2048
                xt = xtp.tile([P, KC * T], F32R, tag="xt")
                xt3 = xt[:].rearrange("p (c t) -> p c t", t=T)
                for tt in range(NTT):
                    xrow = xrowp.tile([P, C], F32, tag="xrow")
                    nc.sync.dma_start(xrow[:], x[tt * P:(tt + 1) * P, :])
                    for cg in range(2):  # groups of 4 transposes per psum tile
                        ptr = pstr.tile([P, 512], F32, tag="ptr")
                        for j in range(4):
                            cc = cg * 4 + j
                            nc.tensor.transpose(
                                ptr[:, j * P:(j + 1) * P],
                                xrow[:, cc * P:(cc + 1) * P],
                                ident[:],
                            )
                        nc.vector.tensor_copy(
                            xt3[:, cg * 4:(cg + 1) * 4, tt * P:(tt + 1) * P],
                            ptr[:].rearrange("p (j t) -> p j t", t=P),
                        )

                # qkv: Yt[n, t] for n-tile 0..11 (Q 0-3, K 4-7, V 8-11)
                for n in range(12):
                    for tc4 in range(4):
                        py = psy.tile([P, QB], F32, tag="py")
                        for kc in range(KC):
                            if tc4 == 0:
                                wt = wtp.tile([P, P], F32R, tag="wt")
                                nc.sync.dma_start(
                                    wt[:],
                                    wqkv[kc * P:(kc + 1) * P, n * P:(n + 1) * P],
                                )
                                if kc == 0:
                                    wts = []
                                wts.append(wt)
                            nc.tensor.matmul(
                                py[:],
                                (wts[kc][:]),
                                (xt3[:, kc, tc4 * QB:(tc4 + 1) * QB]),
                                start=(kc == 0),
                                stop=(kc == KC - 1),
                            )
                        if n < 8:
                            # Q^T/K^T with bias (per-partition scalar add)
                            nc.vector.tensor_scalar_add(
                                yt[:, n * T + tc4 * QB: n * T + (tc4 + 1) * QB],
                                py[:],
                                bq_sb[:, n:n + 1],
                            )
                        else:
                            # V^T chunk -> transpose to natural V in vaug
                            vn = n - 8  # covers heads 2vn, 2vn+1
                            vtmp = vtmpp.tile([P, QB], F32, tag="vtmp")
                            nc.vector.tensor_scalar_add(
                                vtmp[:], py[:], bq_sb[:, n:n + 1]
                            )
                            ptr = pstr.tile([P, 512], F32, tag="ptr")
                            for j in range(4):
                                nc.tensor.transpose(
                                    ptr[:, j * P:(j + 1) * P],
                                    vtmp[:, j * P:(j + 1) * P],
                                    ident[:],
                                )
                            ptr3 = ptr[:].rearrange("p (j t) -> p j t", t=P)
                            for hh in range(2):
                                h = 2 * vn + hh
                                nc.vector.tensor_copy(
                                    vaug3[:, h * NTT + tc4 * 4: h * NTT + tc4 * 4 + 4, 0:64],
                                    ptr3[:, :, hh * 64:hh * 64 + 64],
                                )

            # ---------------- stage 2+3: attention, gather, out proj ----
            with (
                tc.tile_pool(name="pt", bufs=36) as ptp,
                tc.tile_pool(name="recip", bufs=3) as recipp,
                tc.tile_pool(name="bc", bufs=3) as bcp,
                tc.tile_pool(name="atv", bufs=3) as atvp,
                tc.tile_pool(name="w2", bufs=1) as w2p,
                tc.tile_pool(name="agr", bufs=2) as agrp,
                tc.tile_pool(name="outsb", bufs=3) as outsbp,
                tc.tile_pool(name="ps_s", bufs=4, space="PSUM") as pss,
                tc.tile_pool(name="ps_a", bufs=2, space="PSUM") as psa,
                tc.tile_pool(name="ps_o", bufs=2, space="PSUM") as pso,
            ):
                w2sb = w2p.tile([P, KC * CPC], F32R, tag="w2")
                nc.sync.dma_start(
                    w2sb[:].rearrange("p (c n) -> p c n", n=CPC),
                    wout.ap().rearrange("(c p) n -> p c n", p=P),
                )
                w23 = w2sb[:].rearrange("p (c n) -> p c n", n=CPC)

                def s_pass(qb, h):
                    """score matmuls + exp (+causal mask) for one head/qblock.
                    Diagonal k-tiles first so their exp+mask (on the PV
                    critical path) complete while off-diagonal scores stream.
                    """
                    poff = (h % 2) * 64
                    qchunk = h // 2
                    kchunk = 4 + h // 2
                    nkt = 4 * qb + 4
                    kts = list(range(4 * qb, nkt)) + list(range(0, 4 * qb))
                    pts = []
                    for kt in kts:
                        j = kt - 4 * qb  # >=0 on diagonal tiles
                        qoff = max(j, 0) * P
                        ps = pss.tile([P, QB], F32, tag="ps")
                        nc.tensor.matmul(
                            ps[:, qoff:QB],
                            yt[poff:poff + 64,
                               kchunk * T + kt * P: kchunk * T + (kt + 1) * P],
                            yt[poff:poff + 64,
                               qchunk * T + qb * QB + qoff: qchunk * T + (qb + 1) * QB],
                            start=True, stop=True,
                        )
                        pt = ptp.tile([P, QB], BF16, tag="pt")
                        nc.scalar.activation(
                            pt[:, qoff:QB], ps[:, qoff:QB], EXP, scale=SCALE
                        )
                        if j >= 0:
                            # zero where q < k (also fills the stale prefix)
                            nc.gpsimd.affine_select(
                                out=pt[:],
                                in_=pt[:],
                                compare_op=mybir.AluOpType.is_ge,
                                fill=0.0,
                                base=-j * P,
                                pattern=[[1, QB]],
                                channel_multiplier=-1,
                            )
                        pts.append((kt, pt))
                    return pts

                def pv_pass(qb, h, pts, ag_in):
                    pa = psa.tile([P, QB], F32, tag="pa")
                    for i, (kt, pt) in enumerate(pts):
                        nc.tensor.matmul(
                            pa[0:65, :],
                            vaug3[:, h * NTT + kt, :],
                            pt[:],
                            start=(i == 0),
                            stop=(i == len(pts) - 1),
                        )
                    sums = recipp.tile([1, QB], F32, tag="sums")
                    nc.vector.tensor_copy(sums[:], pa[64:65, :])
                    recip = recipp.tile([1, QB], F32, tag="recip")
                    nc.vector.reciprocal_approx_fast(recip[:], sums[:])
                    bc = bcp.tile([64, QB], F32, tag="bc")
                    nc.gpsimd.partition_broadcast(bc[:], recip[:])
                    atv = atvp.tile([64, QB], F32R, tag="atv")
                    nc.vector.tensor_mul(atv[:], pa[0:64, :], bc[:])
                    nc.sync.dma_start(ag_in[h * 64:(h + 1) * 64, :], atv[:])

                def out_proj(qb, ag_out):
                    agr = agrp.tile([P, KC * QB], F32R, tag="agr")
                    nc.sync.dma_start(
                        agr[:].rearrange("p (c n) -> p c n", n=QB),
                        ag_out[:].rearrange("(c p) n -> p c n", p=P),
                    )
                    agr3 = agr[:].rearrange("p (c n) -> p c n", n=QB)
                    for oc in range(4):
                        po = pso.tile([P, QB], F32, tag="po")
                        for cc in range(KC):
                            nc.tensor.matmul(
                                po[:],
                                w23[:, cc, oc * P:(oc + 1) * P],
                                agr3[:, cc, :],
                                start=(cc == 0),
                                stop=(cc == KC - 1),
                            )
                        osb = outsbp.tile([P, QB], F32, tag="osb")
                        nc.vector.tensor_scalar_add(
                            osb[:], po[:], bo_sb[:, oc:oc + 1]
                        )
                        nc.sync.dma_start(
                            outT[oc * P:(oc + 1) * P, qb * QB:(qb + 1) * QB],
                            osb[:],
                        )

                pending_outproj = None
                for qb in range(NQB):
                    ag_in = dramp.tile([CPC, QB], F32R, tag=f"agin{qb}")
                    ag_out = dramp.tile([2 * CPC, QB], F32R, tag=f"agout{qb}")
                    prev = None
                    for h in range(HPC):
                        cur = s_pass(qb, h)
                        if h == 3 and pending_outproj is not None:
                            # emit previous qblock's out-projection here so its
                            # AllGather wait hides behind this qblock's scores
                            pending_outproj()
                            pending_outproj = None
                        if prev is not None:
                            pv_pass(qb, h - 1, prev, ag_in)
                        prev = cur
                    pv_pass(qb, HPC - 1, prev, ag_in)

                    nc.gpsimd.collective_compute(
                        "AllGather",
                        mybir.AluOpType.bypass,
                        replica_groups=groups,
                        ins=[ag_in.opt()],
                        outs=[ag_out.opt()],
                    )
                    pending_outproj = (
                        lambda qb=qb, ag_out=ag_out: out_proj(qb, ag_out)
                    )
                pending_outproj()

    nc.compile()
    return nc


def kernel(x, w_qkv, b_qkv, w_out, b_out):
    x = np.ascontiguousarray(np.asarray(x, dtype=np.float32))
    w_qkv = np.asarray(w_qkv, dtype=np.float32)
    b_qkv = np.asarray(b_qkv, dtype=np.float32)
    w_out = np.asarray(w_out, dtype=np.float32)
    b_out = np.asarray(b_out, dtype=np.float32)

    if "nc" not in _CACHE:
        _CACHE["nc"] = build()
    nc = _CACHE["nc"]

    in_maps = []
    for c in range(NCORES):
        b = c // 2
        h0 = (c % 2) * HPC
        cols = slice(h0 * D, h0 * D + CPC)
        wq = np.concatenate(
            [w_qkv[:, cols], w_qkv[:, C:][:, cols], w_qkv[:, 2 * C:][:, cols]],
            axis=1,
        )
        bq = np.concatenate(
            [b_qkv[cols], b_qkv[C:][cols], b_qkv[2 * C:][cols]]
        ).reshape(1, 3 * CPC)
        half = slice((c % 2) * CPC, (c % 2) * CPC + CPC)
        in_maps.append({
            "x": np.ascontiguousarray(x[b]),
            "wqkv": np.ascontiguousarray(wq),
            "bqkv": np.ascontiguousarray(bq),
            "wout": np.ascontiguousarray(w_out[:, half]),
            "bout": np.ascontiguousarray(b_out[half]).reshape(1, CPC),
        })

    kwargs = {}
    tdir = os.environ.get("KERNEL_TRACE_DIR")
    if tdir:
        kwargs = dict(trace=True, tmpdir=tdir)
    res = run_bass_kernel_spmd(nc, in_maps, core_ids=list(range(NCORES)), **kwargs)
    _CACHE["last_results"] = res

    out = np.empty((B, T, C), dtype=np.float32)
    for c in range(NCORES):
        b = c // 2
        half = slice((c % 2) * CPC, (c % 2) * CPC + CPC)
        out[b][:, half] = res.results[c]["outT"].T
    return out


# revision 15
# speedup vs baseline: 1.4981x; 1.1470x over previous
"""Masked multi-head self-attention kernel for 8 Trainium2 NeuronCores.

Full module: qkv projection -> causal softmax attention (16 heads) -> out
projection, for x[4, 2048, 1024].

Sharding: core c handles batch b = c//2 and heads h0 = (c%2)*8 .. h0+8.
QKV projection + attention are fully local to a core.  The out projection
contracts over all 16 heads' channels, so the two cores of a batch exchange
their attention outputs with a pairwise AllGather (chunked over the query
dim for overlap) and each computes half of the output columns.  Each core
returns out[b][:, half].T (transposed: [512, 2048]); the host reassembles.
"""

import math
import os
import sys

for _p in ("/opt/trn_rl_repo", "/root/.axon_site/_ro/trn_rl_repo"):
    if os.path.isdir(_p) and _p not in sys.path:
        sys.path.insert(0, _p)
        break

import numpy as np

import concourse.bass as bass
import concourse.mybir as mybir
import concourse.tile as tile
from concourse import bacc
from concourse.bass_utils import run_bass_kernel_spmd
from concourse.masks import make_identity

B, T, C, H = 4, 2048, 1024, 16
D = 64                 # head dim
NCORES = 8
HPC = H // 2           # heads per core = 8
CPC = HPC * D          # channels per core = 512
P = 128                # partitions
QB = 512               # query block
NQB = T // QB          # 4
KC = C // P            # contraction chunks for C = 8
NTT = T // P           # 16 t-tiles
SCALE = 1.0 / math.sqrt(D)

F32 = mybir.dt.float32
F32R = mybir.dt.float32r
BF16 = mybir.dt.bfloat16
EXP = mybir.ActivationFunctionType.Exp

_CACHE = {}


def build():
    nc = bacc.Bacc("TRN2", num_devices=NCORES, debug=False)

    x = nc.dram_tensor("x", [T, C], F32, kind="ExternalInput")
    wqkv = nc.dram_tensor("wqkv", [C, 3 * CPC], F32R, kind="ExternalInput")
    bqkv = nc.dram_tensor("bqkv", [1, 3 * CPC], F32, kind="ExternalInput")
    wout = nc.dram_tensor("wout", [C, CPC], F32R, kind="ExternalInput")
    bout = nc.dram_tensor("bout", [1, CPC], F32, kind="ExternalInput")
    outT = nc.dram_tensor("outT", [CPC, T], F32, kind="ExternalOutput")

    groups = [[0, 1], [2, 3], [4, 5], [6, 7]]

    with tile.TileContext(nc) as tc:
        with (
            tc.tile_pool(name="const", bufs=1) as constp,
            tc.tile_pool(name="ytp", bufs=1) as ytp,
            tc.tile_pool(name="vaugp", bufs=1) as vaugp,
            tc.tile_pool(name="dram", bufs=1, space="DRAM") as dramp,
        ):
            ident = constp.tile([P, P], F32, tag="ident")
            make_identity(nc, ident[:])
            # per-partition bias layouts: bq_sb[p, n] = bqkv[n*128 + p]
            bq_sb = constp.tile([P, 12], F32, tag="bq")
            nc.sync.dma_start(
                bq_sb[:].rearrange("p (o n) -> p o n", o=1),
                bqkv.ap().rearrange("o (n p) -> p o n", p=P),
            )
            bo_sb = constp.tile([P, 4], F32, tag="bo")
            nc.sync.dma_start(
                bo_sb[:].rearrange("p (o n) -> p o n", o=1),
                bout.ap().rearrange("o (n p) -> p o n", p=P),
            )

            # Q^T,K^T: chunk n in 0..7 ([128 ch, 2048 t] at cols n*2048)
            yt = ytp.tile([P, 8 * T], BF16, tag="yt")
            # V natural (+ones col): head h, ktile k at cols (h*16+k)*65
            vaug = vaugp.tile([P, HPC * NTT * 65], BF16, tag="vaug")
            vaug3 = vaug[:].rearrange("p (k c) -> p k c", c=65)
            ones_f32 = constp.tile([P, P], F32, tag="ones")
            nc.vector.memset(ones_f32[:], 1.0)
            nc.vector.tensor_copy(
                vaug3[:, :, 64:65],
                ones_f32[:].rearrange("p (a b) -> p a b", b=1),
            )

            # ---------------- stage 1: x^T, qkv projection, V ----------
            with (
                tc.tile_pool(name="xtp", bufs=1) as xtp,
                tc.tile_pool(name="xrow", bufs=3) as xrowp,
                tc.tile_pool(name="wtile", bufs=10) as wtp,
                tc.tile_pool(name="vtmp", bufs=2) as vtmpp,
                tc.tile_pool(name="ps_y", bufs=6, space="PSUM") as psy,
                tc.tile_pool(name="ps_tr", bufs=2, space="PSUM") as pstr,
            ):
                # x^T chunks: [128 ch, 2048 t] at cols cc*2048
                xt = xtp.tile([P, KC * T], F32R, tag="xt")
                xt3 = xt[:].rearrange("p (c t) -> p c t", t=T)
                for tt in range(NTT):
                    xrow = xrowp.tile([P, C], F32, tag="xrow")
                    nc.sync.dma_start(xrow[:], x[tt * P:(tt + 1) * P, :])
                    for cg in range(2):  # groups of 4 transposes per psum tile
                        ptr = pstr.tile([P, 512], F32, tag="ptr")
                        for j in range(4):
                            cc = cg * 4 + j
                            nc.tensor.transpose(
                                ptr[:, j * P:(j + 1) * P],
                                xrow[:, cc * P:(cc + 1) * P],
                                ident[:],
                            )
                        nc.vector.tensor_copy(
                            xt3[:, cg * 4:(cg + 1) * 4, tt * P:(tt + 1) * P],
                            ptr[:].rearrange("p (j t) -> p j t", t=P),
                        )

                # qkv: Yt[n, t] for n-tile 0..11 (Q 0-3, K 4-7, V 8-11)
                # kc outer so each weight tile serves 4 consecutive matmuls
                for n in range(12):
                    pys = [psy.tile([P, QB], F32, name=f"py{n}_{i}", tag="py") for i in range(4)]
                    for kc in range(KC):
                        wt = wtp.tile([P, P], F32R, tag="wt")
                        nc.sync.dma_start(
                            wt[:],
                            wqkv[kc * P:(kc + 1) * P, n * P:(n + 1) * P],
                        )
                        for tc4 in range(4):
                            nc.tensor.matmul(
                                pys[tc4][:],
                                wt[:],
                                xt3[:, kc, tc4 * QB:(tc4 + 1) * QB],
                                start=(kc == 0),
                                stop=(kc == KC - 1),
                            )
                    for tc4 in range(4):
                        py = pys[tc4]
                        if n < 8:
                            # Q^T/K^T with bias (per-partition scalar add)
                            nc.vector.tensor_scalar_add(
                                yt[:, n * T + tc4 * QB: n * T + (tc4 + 1) * QB],
                                py[:],
                                bq_sb[:, n:n + 1],
                            )
                        else:
                            # V^T chunk -> transpose to natural V in vaug
                            vn = n - 8  # covers heads 2vn, 2vn+1
                            vtmp = vtmpp.tile([P, QB], F32, tag="vtmp")
                            nc.vector.tensor_scalar_add(
                                vtmp[:], py[:], bq_sb[:, n:n + 1]
                            )
                            ptr = pstr.tile([P, 512], F32, tag="ptr")
                            for j in range(4):
                                nc.tensor.transpose(
                                    ptr[:, j * P:(j + 1) * P],
                                    vtmp[:, j * P:(j + 1) * P],
                                    ident[:],
                                )
                            ptr3 = ptr[:].rearrange("p (j t) -> p j t", t=P)
                            for hh in range(2):
                                h = 2 * vn + hh
                                nc.vector.tensor_copy(
                                    vaug3[:, h * NTT + tc4 * 4: h * NTT + tc4 * 4 + 4, 0:64],
                                    ptr3[:, :, hh * 64:hh * 64 + 64],
                                )

            # ---------------- stage 2+3: attention, gather, out proj ----
            with (
                tc.tile_pool(name="pt", bufs=36) as ptp,
                tc.tile_pool(name="recip", bufs=3) as recipp,
                tc.tile_pool(name="bc", bufs=3) as bcp,
                tc.tile_pool(name="atv", bufs=3) as atvp,
                tc.tile_pool(name="w2", bufs=1) as w2p,
                tc.tile_pool(name="agr", bufs=2) as agrp,
                tc.tile_pool(name="outsb", bufs=3) as outsbp,
                tc.tile_pool(name="ps_s", bufs=4, space="PSUM") as pss,
                tc.tile_pool(name="ps_a", bufs=2, space="PSUM") as psa,
                tc.tile_pool(name="ps_o", bufs=2, space="PSUM") as pso,
            ):
                w2sb = w2p.tile([P, KC * CPC], F32R, tag="w2")
                nc.sync.dma_start(
                    w2sb[:].rearrange("p (c n) -> p c n", n=CPC),
                    wout.ap().rearrange("(c p) n -> p c n", p=P),
                )
                w23 = w2sb[:].rearrange("p (c n) -> p c n", n=CPC)

                def s_pass(qb, h):
                    """score matmuls + exp (+causal mask) for one head/qblock.
                    Diagonal k-tiles first so their exp+mask (on the PV
                    critical path) complete while off-diagonal scores stream.
                    """
                    poff = (h % 2) * 64
                    qchunk = h // 2
                    kchunk = 4 + h // 2
                    nkt = 4 * qb + 4
                    kts = list(range(4 * qb, nkt)) + list(range(0, 4 * qb))
                    pts = []
                    for kt in kts:
                        j = kt - 4 * qb  # >=0 on diagonal tiles
                        qoff = max(j, 0) * P
                        ps = pss.tile([P, QB], F32, tag="ps")
                        nc.tensor.matmul(
                            ps[:, qoff:QB],
                            yt[poff:poff + 64,
                               kchunk * T + kt * P: kchunk * T + (kt + 1) * P],
                            yt[poff:poff + 64,
                               qchunk * T + qb * QB + qoff: qchunk * T + (qb + 1) * QB],
                            start=True, stop=True,
                        )
                        pt = ptp.tile([P, QB], BF16, tag="pt")
                        nc.scalar.activation(
                            pt[:, qoff:QB], ps[:, qoff:QB], EXP, scale=SCALE
                        )
                        if j >= 0:
                            # zero where q < k (also fills the stale prefix)
                            nc.gpsimd.affine_select(
                                out=pt[:],
                                in_=pt[:],
                                compare_op=mybir.AluOpType.is_ge,
                                fill=0.0,
                                base=-j * P,
                                pattern=[[1, QB]],
                                channel_multiplier=-1,
                            )
                        pts.append((kt, pt))
                    return pts

                def pv_pass(qb, h, pts, ag_in):
                    pa = psa.tile([P, QB], F32, tag="pa")
                    for i, (kt, pt) in enumerate(pts):
                        nc.tensor.matmul(
                            pa[0:65, :],
                            vaug3[:, h * NTT + kt, :],
                            pt[:],
                            start=(i == 0),
                            stop=(i == len(pts) - 1),
                        )
                    sums = recipp.tile([1, QB], F32, tag="sums")
                    nc.vector.tensor_copy(sums[:], pa[64:65, :])
                    recip = recipp.tile([1, QB], F32, tag="recip")
                    nc.vector.reciprocal_approx_fast(recip[:], sums[:])
                    bc = bcp.tile([64, QB], F32, tag="bc")
                    nc.gpsimd.partition_broadcast(bc[:], recip[:])
                    atv = atvp.tile([64, QB], F32R, tag="atv")
                    nc.vector.tensor_mul(atv[:], pa[0:64, :], bc[:])
                    nc.sync.dma_start(ag_in[(h % 4) * 64:(h % 4) * 64 + 64, :], atv[:])

                def gather(ag_in, ag_out):
                    nc.gpsimd.collective_compute(
                        "AllGather",
                        mybir.AluOpType.bypass,
                        replica_groups=groups,
                        ins=[ag_in.opt()],
                        outs=[ag_out.opt()],
                    )

                def out_proj(qb, ag_outs):
                    # w_out rows are host-permuted to match the gathered
                    # row order [even03, odd03, even47, odd47]
                    agr3s = []
                    for ago in ag_outs:
                        agr = agrp.tile([P, 4 * QB], F32R, tag="agr")
                        nc.sync.dma_start(
                            agr[:].rearrange("p (c n) -> p c n", n=QB),
                            ago[:].rearrange("(c p) n -> p c n", p=P),
                        )
                        agr3s.append(agr[:].rearrange("p (c n) -> p c n", n=QB))
                    for oc in range(4):
                        po = pso.tile([P, QB], F32, tag="po")
                        for cc in range(KC):
                            nc.tensor.matmul(
                                po[:],
                                w23[:, cc, oc * P:(oc + 1) * P],
                                agr3s[cc // 4][:, cc % 4, :],
                                start=(cc == 0),
                                stop=(cc == KC - 1),
                            )
                        osb = outsbp.tile([P, QB], F32, tag="osb")
                        nc.vector.tensor_scalar_add(
                            osb[:], po[:], bo_sb[:, oc:oc + 1]
                        )
                        nc.sync.dma_start(
                            outT[oc * P:(oc + 1) * P, qb * QB:(qb + 1) * QB],
                            osb[:],
                        )

                pending_outproj = None
                for qb in range(NQB):
                    # two half-gathers per qblock: heads 0-3 gather while
                    # heads 4-7 attention still runs
                    ag_ins = [
                        dramp.tile([CPC // 2, QB], F32R, name=f"agin{qb}_{i}", tag=f"agin{qb}_{i}")
                        for i in range(2)
                    ]
                    ag_outs = [
                        dramp.tile([CPC, QB], F32R, name=f"agout{qb}_{i}", tag=f"agout{qb}_{i}")
                        for i in range(2)
                    ]
                    prev = None
                    for h in range(HPC):
                        cur = s_pass(qb, h)
                        if h == 3 and pending_outproj is not None:
                            # emit previous qblock's out-projection here so its
                            # AllGather wait hides behind this qblock's scores
                            pending_outproj()
                            pending_outproj = None
                        if prev is not None:
                            pv_pass(qb, h - 1, prev, ag_ins[(h - 1) // 4])
                            if h - 1 == 3:
                                gather(ag_ins[0], ag_outs[0])
                        prev = cur
                    pv_pass(qb, HPC - 1, prev, ag_ins[1])
                    gather(ag_ins[1], ag_outs[1])
                    pending_outproj = (
                        lambda qb=qb, ag_outs=ag_outs: out_proj(qb, ag_outs)
                    )
                pending_outproj()

    nc.compile()
    return nc


def kernel(x, w_qkv, b_qkv, w_out, b_out):
    x = np.ascontiguousarray(np.asarray(x, dtype=np.float32))
    w_qkv = np.asarray(w_qkv, dtype=np.float32)
    b_qkv = np.asarray(b_qkv, dtype=np.float32)
    w_out = np.asarray(w_out, dtype=np.float32)
    b_out = np.asarray(b_out, dtype=np.float32)

    if "nc" not in _CACHE:
        _CACHE["nc"] = build()
    nc = _CACHE["nc"]

    in_maps = []
    for c in range(NCORES):
        b = c // 2
        h0 = (c % 2) * HPC
        cols = slice(h0 * D, h0 * D + CPC)
        wq = np.concatenate(
            [w_qkv[:, cols], w_qkv[:, C:][:, cols], w_qkv[:, 2 * C:][:, cols]],
            axis=1,
        )
        bq = np.concatenate(
            [b_qkv[cols], b_qkv[C:][cols], b_qkv[2 * C:][cols]]
        ).reshape(1, 3 * CPC)
        half = slice((c % 2) * CPC, (c % 2) * CPC + CPC)
        wo = w_out[:, half]
        wo_perm = np.concatenate(
            [wo[0:256], wo[512:768], wo[256:512], wo[768:1024]], axis=0
        )
        in_maps.append({
            "x": np.ascontiguousarray(x[b]),
            "wqkv": np.ascontiguousarray(wq),
            "bqkv": np.ascontiguousarray(bq),
            "wout": np.ascontiguousarray(wo_perm),
            "bout": np.ascontiguousarray(b_out[half]).reshape(1, CPC),
        })

    kwargs = {}
    tdir = os.environ.get("KERNEL_TRACE_DIR")
    if tdir:
        kwargs = dict(trace=True, tmpdir=tdir)
    res = run_bass_kernel_spmd(nc, in_maps, core_ids=list(range(NCORES)), **kwargs)
    _CACHE["last_results"] = res

    out = np.empty((B, T, C), dtype=np.float32)
    for c in range(NCORES):
        b = c // 2
        half = slice((c % 2) * CPC, (c % 2) * CPC + CPC)
        out[b][:, half] = res.results[c]["outT"].T
    return out
